# revision 1
# baseline (speedup 1.0000x reference)
"""Trainium2 Bass kernel for nn_CausalAttention (b=2, t=2048, d=2048, 16 heads).

Strategy (8 NeuronCores, SPMD):
  - Head-sharded QKV projections + attention: core c owns global heads 2c, 2c+1
    and computes q/k/v (RoPE applied to q,k) plus causal softmax-attention for
    those heads over ALL 4096 (b,t) rows. Fully local, no collective needed.
  - Logits are computed transposed ([keys, q]) so no on-chip transposes are
    needed anywhere: qT/kT land in [head_dim, rows] layout straight from the
    projection matmuls, and v is produced in natural [rows, head_dim] layout.
  - Softmax denominators via a ones-vector matmul (partition-dim reduction on
    the PE), reciprocal on DVE, partition-broadcast on GPSIMD.
  - Two AllToAlls (one per local head, the first overlapped with the second
    head's attention) redistribute attention outputs from head-sharded to
    row-sharded; each core then runs the output projection (full Wo) for its
    512 rows. Host concatenates the 8 row-shards.
  - All matmuls in bf16 with fp32 PSUM accumulation; softmax stats in fp32.
"""

import numpy as np
import ml_dtypes

import concourse.bass as bass
import concourse.tile as tile
import concourse.mybir as mybir
from concourse import bacc
from concourse.bass_utils import run_bass_kernel_spmd

BF16 = ml_dtypes.bfloat16
DT_BF = mybir.dt.bfloat16
DT_F32 = mybir.dt.float32

NCORES = 8
B, T, D = 2, 2048, 2048
H, HD = 16, D // 16          # 16 heads, head_dim 128
HLOC = H // NCORES           # 2 heads per core
ROWS = B * T                 # 4096 flattened rows
RPC = ROWS // NCORES         # 512 rows per core (for output projection)
NT = ROWS // 512             # 8 row-tiles of 512 for QKV phase
KCH = D // 128               # 16 contraction chunks
QCH = 512                    # query chunk (free dim of attention matmuls)
NQC = T // QCH               # 4 query chunks per (batch, head)
NDIAG = QCH // 128           # 4 diagonal key blocks per query chunk
SCALE = 1.0 / float(np.sqrt(HD))

_CACHE = {}


def _build(with_bias, reps=1, only_phase=None):
    nc = bacc.Bacc("TRN2", target_bir_lowering=False, debug=False,
                   num_devices=NCORES)

    xT_d = nc.declare_dram_parameter("xT", [D, ROWS], DT_BF, isOutput=False)
    wq_d = nc.declare_dram_parameter("wq", [D, HLOC * HD], DT_BF, isOutput=False)
    wk_d = nc.declare_dram_parameter("wk", [D, HLOC * HD], DT_BF, isOutput=False)
    wv_d = nc.declare_dram_parameter("wv", [D, HLOC * HD], DT_BF, isOutput=False)
    wo_d = nc.declare_dram_parameter("wo", [D, D], DT_BF, isOutput=False)
    cos_d = nc.declare_dram_parameter("cosT", [HD, T], DT_BF, isOutput=False)
    sins_d = nc.declare_dram_parameter("sinsT", [HD, T], DT_BF, isOutput=False)
    mk_d = nc.declare_dram_parameter("maskT", [NDIAG, 128, QCH], DT_BF,
                                     isOutput=False)
    if with_bias:
        bqk_d = nc.declare_dram_parameter("bqk", [2, HLOC * HD], DT_BF, isOutput=False)
        bv_d = nc.declare_dram_parameter("bvs", [1, HLOC * HD], DT_BF, isOutput=False)
        bo_d = nc.declare_dram_parameter("bos", [1, D], DT_BF, isOutput=False)
    out_d = nc.declare_dram_parameter("out", [RPC, D], DT_F32, isOutput=True)

    with tile.TileContext(nc) as tc:
        with (
            tc.tile_pool(name="singles", bufs=1) as singles,
            tc.tile_pool(name="xt", bufs=2) as xt_pool,
            tc.tile_pool(name="slabs", bufs=1) as slabs,
            tc.tile_pool(name="rope", bufs=2) as rope_pool,
            tc.tile_pool(name="expp", bufs=4) as exp_pool,
            tc.tile_pool(name="attn", bufs=2) as attn_pool,
            tc.tile_pool(name="rcp", bufs=2) as rcp_pool,
            tc.tile_pool(name="wop", bufs=2) as wo_pool,
            tc.tile_pool(name="osb", bufs=2) as out_pool,
            tc.tile_pool(name="evn", bufs=6) as evn_pool,
            tc.tile_pool(name="psA", bufs=2, space="PSUM") as psA,
            tc.tile_pool(name="psL", bufs=2, space="PSUM") as psL,
            tc.tile_pool(name="psO", bufs=2, space="PSUM") as psO,
            tc.tile_pool(name="psD", bufs=2, space="PSUM") as psD,
            tc.tile_pool(name="dram", bufs=1, space="DRAM") as dram,
        ):
            # ---- resident constants -------------------------------------
            wq_sb = singles.tile([128, KCH, HLOC * HD], DT_BF, tag="wq", name="wq")
            wk_sb = singles.tile([128, KCH, HLOC * HD], DT_BF, tag="wk", name="wk")
            wv_sb = singles.tile([128, KCH, HLOC * HD], DT_BF, tag="wv", name="wv")
            wq_ap_full = wq_d.ap().rearrange("(k p) c -> p k c", p=128)
            cos_sb = singles.tile([HD, T], DT_BF, tag="cos", name="cos")
            sins_sb = singles.tile([HD, T], DT_BF, tag="sins", name="sins")
            mk_sb = singles.tile([128, NDIAG, QCH], DT_BF, tag="mk", name="mk")
            ones_sb = singles.tile([128, 1], DT_BF, tag="ones", name="ones")
            nc.vector.memset(ones_sb, 1.0)
            ones_r128 = singles.tile([1, 128], DT_BF, tag="ones1", name="ones1")
            nc.vector.memset(ones_r128, 1.0)
            if with_bias:
                bqk_sb = singles.tile([2, HLOC * HD], DT_BF, tag="bqk", name="bqk")
                bv_sb = singles.tile([1, HLOC * HD], DT_BF, tag="bv", name="bv")
                bo_sb = singles.tile([1, D], DT_BF, tag="bo", name="bo")
                ones_row = singles.tile([1, 512], DT_BF, tag="onesrow", name="onesrow")
                nc.sync.dma_start(out=bqk_sb, in_=bqk_d.ap())
                nc.sync.dma_start(out=bv_sb, in_=bv_d.ap())
                nc.sync.dma_start(out=bo_sb, in_=bo_d.ap())
                nc.vector.memset(ones_row, 1.0)

            # persistent per-head slabs ([head_dim, rows] for q/k; natural for v)
            q_slab = [slabs.tile([HD, ROWS], DT_BF, tag=f"qs{m}", name=f"qs{m}")
                      for m in range(HLOC)]
            k_slab = [slabs.tile([HD, ROWS], DT_BF, tag=f"ks{m}", name=f"ks{m}")
                      for m in range(HLOC)]
            v_slab = slabs.tile([128, ROWS // 128, HLOC * HD], DT_BF, tag="vs",
                                name="vs")
            ao_sb = slabs.tile([128, NCORES, HLOC, RPC], DT_BF, tag="ao", name="ao")

            if only_phase in (2, 3):
                for sl_ in q_slab + k_slab:
                    nc.vector.memset(sl_, 0.0)
                nc.vector.memset(v_slab, 0.0)

            a2a_in = [dram.tile([NCORES, 128, RPC], DT_BF, name=f"a2ai{m}")
                      for m in range(HLOC)]
            a2a_out = [dram.tile([NCORES, 128, RPC], DT_BF, name=f"a2ao{m}")
                       for m in range(HLOC)]

            xT_ap = xT_d.ap().rearrange("(k p) r -> p k r", p=128)

            xt_first = None
            if only_phase in (None, 1):
                xt_first = xt_pool.tile([128, KCH, 512], DT_BF, tag="xt",
                                        name="xtf")
                for kc in range(KCH):
                    # one k-chunk of x then the matching k-chunk of Wq, queued
                    # FIRST so the opening accumulation group streams while
                    # the rest of the constants drain behind it
                    nc.sync.dma_start(
                        out=xt_first[:, kc:kc + 1, :],
                        in_=xT_ap[:, kc:kc + 1, 0:512])
                    nc.sync.dma_start(out=wq_sb[:, kc:kc + 1, :],
                                      in_=wq_ap_full[:, kc:kc + 1, :])
            # remaining constants, in order of first use (wk for the k-proj,
            # cos/sins for RoPE, wv for the v-proj, mask only at attention)
            for w_sb_, w_d_ in ((wk_sb, wk_d), (wv_sb, wv_d)):
                w_ap_ = w_d_.ap().rearrange("(k p) c -> p k c", p=128)
                for kc in range(4):
                    nc.sync.dma_start(out=w_sb_[:, 4 * kc:4 * (kc + 1), :],
                                      in_=w_ap_[:, 4 * kc:4 * (kc + 1), :])
            nc.sync.dma_start(out=cos_sb, in_=cos_d.ap())
            nc.sync.dma_start(out=sins_sb, in_=sins_d.ap())
            nc.sync.dma_start(out=mk_sb, in_=mk_d.ap().rearrange("d p q -> p d q"))

            for _rep in range(reps):
              # ---- phase 1: QKV projections + RoPE ----------------------
              for n in range(NT if only_phase in (None, 1) else 0):
                if n == 0 and _rep == 0 and xt_first is not None:
                    xt = xt_first
                else:
                    xt = xt_pool.tile([128, KCH, 512], DT_BF, tag="xt", name="xt")
                    nc.sync.dma_start(out=xt, in_=xT_ap[:, :, n * 512:(n + 1) * 512])
                tc0 = (n * 512) % T  # position-table column offset
                for m in range(HLOC):
                    for which, w_sb, slab in ((0, wq_sb, q_slab[m]), (1, wk_sb, k_slab[m])):
                        ps = psA.tile([128, 512], DT_F32, tag="psA", name="psA")
                        for k in range(KCH):
                            nc.tensor.matmul(
                                ps, w_sb[:, k, m * HD:(m + 1) * HD], xt[:, k, :],
                                start=(k == 0), stop=(k == KCH - 1 and not with_bias))
                        if with_bias:
                            nc.tensor.matmul(
                                ps, bqk_sb[which:which + 1, m * HD:(m + 1) * HD],
                                ones_row, start=False, stop=True)
                        # RoPE: q' = q*cos + swap64(q)*sins  (sins sign-folded)
                        qf = rope_pool.tile([128, 512], DT_BF, tag="qf", name="qf")
                        nc.scalar.copy(qf, ps)
                        swp = rope_pool.tile([128, 512], DT_BF, tag="swp", name="swp")
                        nc.vector.tensor_copy(swp[0:64, :], qf[64:128, :])
                        nc.vector.tensor_copy(swp[64:128, :], qf[0:64, :])
                        dst = slab[:, n * 512:(n + 1) * 512]
                        nc.vector.tensor_mul(dst, qf, cos_sb[:, tc0:tc0 + 512])
                        t2 = rope_pool.tile([128, 512], DT_BF, tag="t2", name="t2")
                        nc.vector.tensor_mul(t2, swp, sins_sb[:, tc0:tc0 + 512])
                        nc.vector.tensor_add(dst, dst, t2)
                for sub in range(4):
                    ps = psL.tile([128, QCH], DT_F32, tag="psL", name="psL")
                    pv = ps[:, 0:HLOC * HD]
                    for k in range(KCH):
                        nc.tensor.matmul(
                            pv, xt[:, k, sub * 128:(sub + 1) * 128], wv_sb[:, k, :],
                            start=(k == 0), stop=(k == KCH - 1 and not with_bias))
                    if with_bias:
                        nc.tensor.matmul(pv, ones_row[0:1, 0:128], bv_sb,
                                         start=False, stop=True)
                    nc.scalar.copy(v_slab[:, n * 4 + sub, :], pv)

              # ---- phase 2: causal attention (transposed logits) --------
              # m-outer so head m's AllToAll overlaps head m+1's attention.
              # The two batches share the same causal structure per query
              # chunk j, so their units are interleaved at key-block
              # granularity: two independent mm->exp->mm chains keep PE and
              # ACT busy across each other's latency bubbles.
              for m in range(HLOC if only_phase in (None, 2) else 0):
                jseq = list(range(NQC)) if m == 0 else list(range(NQC - 1, -1, -1))
                lctr = 0
                for j in jseq:
                    ps_o = {}
                    ps_den = {}
                    for b in range(B):
                        ps_o[b] = psO.tile([HD, QCH], DT_F32, tag="psO", name="psO")
                        ps_den[b] = psD.tile([1, QCH], DT_F32, tag="psD", name="psD")
                    nkb = NDIAG * (j + 1)
                    for kb in range(nkb):
                        for b in range(B):
                            col0 = b * T
                            q_rhs = q_slab[m][:, col0 + j * QCH: col0 + (j + 1) * QCH]
                            kcol = col0 + kb * 128
                            lpool = psL if lctr % 2 == 0 else psA
                            ps_l = lpool.tile([128, QCH], DT_F32,
                                              tag="psL" if lctr % 2 == 0 else "psA",
                                              name="psl")
                            lctr += 1
                            nc.tensor.matmul(
                                ps_l, k_slab[m][:, kcol:kcol + 128], q_rhs,
                                start=True, stop=True)
                            e = exp_pool.tile([128, QCH], DT_BF, tag="e", name="e")
                            nc.scalar.activation(
                                e, ps_l, mybir.ActivationFunctionType.Exp,
                                scale=SCALE)
                            di = kb - NDIAG * j
                            if di >= 0:
                                # multiplicative 0/1 causal mask, bf16 on SBUF
                                nc.vector.tensor_mul(e, e, mk_sb[:, di, :])
                            nc.tensor.matmul(
                                ps_o[b], v_slab[:, (col0 // 128) + kb, m * HD:(m + 1) * HD],
                                e, start=(kb == 0), stop=(kb == nkb - 1))
                            nc.tensor.matmul(
                                ps_den[b], ones_sb, e,
                                start=(kb == 0), stop=(kb == nkb - 1))
                    for b in range(B):
                        rc = rcp_pool.tile([1, QCH], DT_F32, tag="rc", name="rc")
                        nc.vector.reciprocal(rc, ps_den[b])
                        rcb = rcp_pool.tile([128, QCH], DT_F32, tag="rcb", name="rcb")
                        nc.gpsimd.partition_broadcast(rcb, rc)
                        ac = attn_pool.tile([HD, QCH], DT_BF, tag="ac", name="ac")
                        nc.vector.tensor_mul(ac, ps_o[b], rcb)
                        dest = b * NQC + j
                        nc.sync.dma_start(out=a2a_in[m][dest, :, :], in_=ac)
                if only_phase is None or only_phase == 2:
                    nc.gpsimd.collective_compute(
                        "AllToAll", mybir.AluOpType.bypass,
                        replica_groups=[list(range(NCORES))],
                        ins=[a2a_in[m][:, :, :].opt()],
                        outs=[a2a_out[m][:, :, :].opt()],
                    )
                    # ao loads for m=1 are deferred into phase 3 (after the
                    # even pass): queueing them here would place them ahead of
                    # the even pass's Wo loads on the sync queue, and the
                    # cumulative DMA semaphore would gate the even pass on
                    # this AllToAll's completion.
                    if m == 0 or only_phase == 2:
                        for src in range(NCORES):
                            nc.sync.dma_start(
                                out=ao_sb[:, src, m, :],
                                in_=a2a_out[m][src, :, :])

              # ---- phase 3: output projection ---------------------------
              # Split each (panel, row-tile) into an even-head half (m=0 data,
              # available after the first AllToAll — runs concurrently with the
              # second AllToAll, partial saved to SBUF) and an odd-head half
              # (m=1 data) combined on the DVE.
              if only_phase not in (None, 3):
                continue
              if only_phase == 3:
                nc.vector.memset(ao_sb, 0.0)
              wo_ap = wo_d.ap().rearrange("(h p) c -> p h c", p=128)
              NPAN = 4
              PAN = D // NPAN  # 512-column panels of Wo
              # even pass first for EVERY (panel, row-tile): the PE is
              # in-order, so all even-head work must precede any odd-head
              # (2nd-AllToAll-dependent) instruction in the PE stream for it
              # to fill the collective window. Wo panels are re-loaded in the
              # odd pass (DMA bandwidth is cheap here, PSUM/SBUF slots are not).
              ev_tiles = {}
              for pan in range(NPAN):
                wo_sb = wo_pool.tile([128, H, PAN], DT_BF, tag="wo", name="wo")
                nc.sync.dma_start(
                    out=wo_sb, in_=wo_ap[:, :, pan * PAN:(pan + 1) * PAN])
                for rt in range(RPC // 128):
                    pse = psA.tile([128, 512], DT_F32, tag="psA", name="psA") \
                        if rt % 2 == 0 else \
                        psL.tile([128, 512], DT_F32, tag="psL", name="psL")
                    for i, hs in enumerate(range(NCORES)):
                        nc.tensor.matmul(
                            pse, ao_sb[:, hs, 0, rt * 128:(rt + 1) * 128],
                            wo_sb[:, 2 * hs, :],
                            start=(i == 0), stop=(i == NCORES - 1))
                    ev = evn_pool.tile([128, PAN], DT_BF, tag="ev",
                                       name="ev", bufs=16)
                    nc.scalar.copy(ev, pse)
                    ev_tiles[(pan, rt)] = ev
              if only_phase is None:
                for src in range(NCORES):
                    nc.sync.dma_start(
                        out=ao_sb[:, src, 1, :],
                        in_=a2a_out[1][src, :, :])
              for pan in range(NPAN):
                wo_sb = wo_pool.tile([128, H, PAN], DT_BF, tag="wo", name="wo")
                nc.sync.dma_start(
                    out=wo_sb, in_=wo_ap[:, :, pan * PAN:(pan + 1) * PAN])
                for rt in range(RPC // 128):
                    pso = psO.tile([128, 512], DT_F32, tag="psO", name="psO") \
                        if rt % 2 == 0 else \
                        psA.tile([128, 512], DT_F32, tag="psA", name="psA")
                    for i, hs in enumerate(range(NCORES)):
                        nc.tensor.matmul(
                            pso, ao_sb[:, hs, 1, rt * 128:(rt + 1) * 128],
                            wo_sb[:, 2 * hs + 1, :],
                            start=(i == 0), stop=(i == NCORES - 1 and not with_bias))
                    if with_bias:
                        nc.tensor.matmul(
                            pso, ones_row[0:1, 0:128],
                            bo_sb[:, pan * PAN:(pan + 1) * PAN],
                            start=False, stop=True)
                    o_sb = out_pool.tile([128, PAN], DT_F32, tag="osb", name="osb")
                    nc.vector.tensor_add(o_sb, pso[:, 0:PAN], ev_tiles[(pan, rt)])
                    nc.sync.dma_start(
                        out=out_d[rt * 128:(rt + 1) * 128,
                                  pan * PAN:(pan + 1) * PAN],
                        in_=o_sb)
    nc.compile()
    return nc


def _host_prep(x, mask, Wq, bq, Wk, bk, Wv, bv, Wo, bo, pos_offset):
    x = np.asarray(x, dtype=np.float32)
    mask = np.asarray(mask, dtype=np.float32)
    off = float(np.asarray(pos_offset))
    half = HD // 2

    xT = np.ascontiguousarray(x.reshape(ROWS, D).T).astype(BF16)

    inv_freq = 1.0 / 10000 ** (np.arange(half, dtype=np.float32) / half)
    pos = np.arange(T, dtype=np.float32) + off
    freqs = pos[:, None] * inv_freq[None, :]
    freqs = np.concatenate([freqs, freqs], axis=-1)      # [T, HD]
    cosT = np.ascontiguousarray(np.cos(freqs).T)         # [HD, T]
    sinT = np.sin(freqs).T
    sinsT = np.ascontiguousarray(
        np.concatenate([-sinT[:half], sinT[half:]], axis=0))

    # diagonal-block mask tiles, transposed to [key, q], pre-scaled by sqrt(HD)
    # (the 1/sqrt(HD) softmax scale is folded into the Exp activation).
    m2 = mask[0, 0]                                      # [tq, tk]
    maskT = np.stack([
        np.ascontiguousarray((m2[0:QCH, 128 * i:128 * (i + 1)].T >= 0.0))
        for i in range(NDIAG)
    ]).astype(np.float32)                                # [NDIAG, 128, QCH] 0/1

    Wqb = np.asarray(Wq, np.float32).astype(BF16)
    Wkb = np.asarray(Wk, np.float32).astype(BF16)
    Wvb = np.asarray(Wv, np.float32).astype(BF16)
    Wob = np.ascontiguousarray(np.asarray(Wo, np.float32)).astype(BF16)

    bq = np.asarray(bq, np.float32)
    bk = np.asarray(bk, np.float32)
    bv = np.asarray(bv, np.float32)
    bo = np.asarray(bo, np.float32)
    with_bias = bool(np.any(bq) or np.any(bk) or np.any(bv) or np.any(bo))

    in_maps = []
    for c in range(NCORES):
        sl = slice(c * HLOC * HD, (c + 1) * HLOC * HD)
        m = {
            "xT": xT,
            "wq": np.ascontiguousarray(Wqb[:, sl]),
            "wk": np.ascontiguousarray(Wkb[:, sl]),
            "wv": np.ascontiguousarray(Wvb[:, sl]),
            "wo": Wob,
            "cosT": cosT.astype(BF16),
            "sinsT": sinsT.astype(BF16),
            "maskT": maskT.astype(BF16),
        }
        if with_bias:
            m["bqk"] = np.stack([bq[sl], bk[sl]]).astype(BF16)
            m["bvs"] = bv[sl][None, :].astype(BF16)
            m["bos"] = bo[None, :].astype(BF16)
        in_maps.append(m)
    return in_maps, with_bias


def kernel(x, mask, Wq, bq, Wk, bk, Wv, bv, Wo, bo, pos_offset, _trace=False):
    in_maps, with_bias = _host_prep(
        x, mask, Wq, bq, Wk, bk, Wv, bv, Wo, bo, pos_offset)
    key = with_bias
    if key not in _CACHE:
        _CACHE[key] = _build(with_bias)
    nc = _CACHE[key]
    res = run_bass_kernel_spmd(nc, in_maps, core_ids=list(range(NCORES)),
                               trace=_trace)
    kernel.last_results = res
    out = np.concatenate([res.results[c]["out"] for c in range(NCORES)],
                         axis=0).reshape(B, T, D).astype(np.float32)
    return out

